# revision 28
# baseline (speedup 1.0000x reference)
"""Trainium2 Bass kernel for nn_MAPLoss (smooth-AP loss, N=512, D=256, K=0.001).

v6: bf16 PE pipeline + minimal-DMA layout. The loss reads prec[i] only at
positive (query, item) pairs (~3600 of 512*511), so each core evaluates just
its ~450 pairs, bin-packed row-atomically into [128 x 512] blocks:
  - norms via DVE square + ones-matmul (no per-chunk ACT chain),
    Sqrt on the one act table that also holds Square, then
    reciprocal_approx_fast on DVE; inv broadcast via a fp32r matmul and
    the 64 row-invs via a PE transpose,
  - R (normalized Gram) built with one fused scalar_tensor_tensor and
    stored bf16; per block a bf16 replication matmul (1 cycle/row),
  - bias[p] = -1000*rg[p] accumulated directly by a fused STT against a
    gpsimd-precomputed onehot(sel) mask; the same bias drives both the
    row sigmoid (den, via accum_out) and the pair-pair sigmoid (acc),
  - epilogue: prec = (acc+0.5)/(den+0.5-sigmoid(1000-1000*rg)), weighted
    global sum with host-folded weights w = 1/npos.
Exactly two ACT_TABLE_LOADs (sqrt-set, sigmoid-set), both scheduled while
DMA/PE work proceeds. HBM per core ~550KB: qt in bf16 once, one-hot
metadata in bf16, iota generated on device.
Host passes only index metadata (selectors, one-hots, weights) derived
from `target`; all float FLOPs run on device. Each core returns its
partial numerator; the host sums and finishes 1 - num/cnt.
"""

import numpy as np
from contextlib import ExitStack

N = 512
D = 256
NCORES = 8
RPC = N // NCORES   # rows per core = 64
SLOTS = 16          # max positives per row (max npos observed is 13)
KINV = 1000.0       # 1/K


def _build_program(nblk):
    import concourse.bacc as bacc
    import concourse.tile as tile
    import concourse.mybir as mybir

    fp32 = mybir.dt.float32
    fp32r = mybir.dt.float32r
    bf16 = mybir.dt.bfloat16
    ALU = mybir.AluOpType
    ACT = mybir.ActivationFunctionType
    AX = mybir.AxisListType

    NDC = D // 128          # 2 dim chunks of qT
    BDGS0 = 0               # meta16 column offsets
    IBS0 = 128 * nblk
    MG0 = 144 * nblk
    M16 = 160 * nblk

    nc = bacc.Bacc("TRN2", target_bir_lowering=False, debug=False,
                   num_devices=NCORES)
    qt2_dram = nc.dram_tensor("qt2", [128, NDC * N], bf16,
                              kind="ExternalInput").ap()
    rep_dram = nc.dram_tensor("rep", [RPC, 128 * nblk], bf16,
                              kind="ExternalInput").ap()
    m16_dram = nc.dram_tensor("m16", [128, M16], bf16,
                              kind="ExternalInput").ap()
    m32_dram = nc.dram_tensor("m32", [128, 2 * nblk], fp32,
                              kind="ExternalInput").ap()
    out_dram = nc.dram_tensor("out", [1, 1], fp32, kind="ExternalOutput").ap()

    with tile.TileContext(nc) as tc, ExitStack() as ctx:
        const = ctx.enter_context(tc.tile_pool(name="const", bufs=1))
        persist = ctx.enter_context(tc.tile_pool(name="persist", bufs=1))
        setup_ctx = ctx.enter_context(ExitStack())
        spsum = setup_ctx.enter_context(
            tc.tile_pool(name="spsum", bufs=1, space="PSUM"))
        ssb = setup_ctx.enter_context(tc.tile_pool(name="ssb", bufs=1))

        # --- constants (gpsimd) + the three input DMAs on separate rings ---
        ones_colb = const.tile([128, 1], bf16, tag="ones_colb")
        nc.gpsimd.memset(ones_colb[:], 1.0)
        ones_64b = const.tile([1, RPC], bf16, tag="ones_64b")
        nc.gpsimd.memset(ones_64b[:], 1.0)
        ones_red = const.tile([128, 1], fp32, tag="ones_red")
        nc.gpsimd.memset(ones_red[:], 1.0)
        one_1x1 = const.tile([1, 1], fp32, tag="one_1x1")
        nc.gpsimd.memset(one_1x1[:], 1.0)
        k1000 = const.tile([128, 1], fp32, tag="k1000")
        nc.gpsimd.memset(k1000[:], KINV)
        iota_f = const.tile([128, N], fp32, tag="iota_f")
        nc.gpsimd.iota(iota_f[:], pattern=[[1, N]], base=0,
                       channel_multiplier=0,
                       allow_small_or_imprecise_dtypes=True)

        # DMA throughput is packet-bound (~45ns per partition-row packet
        # regardless of 1KB vs 2KB), so keep full 2KB rows and split by
        # partition range across two queues.
        qt2 = persist.tile([128, NDC * N], bf16, tag="qt2")
        nc.sync.dma_start(qt2[0:64, :], qt2_dram[0:64, :])
        nc.scalar.dma_start(qt2[64:128, :], qt2_dram[64:128, :])
        m16 = persist.tile([128, M16], bf16, tag="m16")
        nc.sync.dma_start(m16[0:64, :], m16_dram[0:64, :])
        nc.scalar.dma_start(m16[64:128, :], m16_dram[64:128, :])
        m32 = persist.tile([128, 2 * nblk], fp32, tag="m32")
        nc.gpsimd.dma_start(m32[:], m32_dram)
        rep = persist.tile([RPC, 128 * nblk], bf16, tag="rep")
        nc.gpsimd.dma_start(rep[:], rep_dram)

        # --- norms: sumsq[1,N] = ones^T (qt.^2), via matmul over NDC chunks ---
        qtsq = ssb.tile([128, NDC * N], bf16, tag="qtsq")
        sumsq_ps = spsum.tile([1, N], fp32, tag="sumsq_ps")
        g_ps = spsum.tile([RPC, N], fp32, tag="g_ps")
        for c in range(NDC):
            nc.vector.tensor_mul(qtsq[:, N * c:N * (c + 1)],
                                 qt2[:, N * c:N * (c + 1)],
                                 qt2[:, N * c:N * (c + 1)])
            nc.tensor.matmul(sumsq_ps[:], ones_colb[:],
                             qtsq[:, N * c:N * (c + 1)],
                             start=(c == 0), stop=(c == NDC - 1))
            # Gram for rows 0..RPC-1 (PE, interleaved with the norm matmuls)
            nc.tensor.matmul(g_ps[:], qt2[:, N * c:N * c + RPC],
                             qt2[:, N * c:N * (c + 1)],
                             start=(c == 0), stop=(c == NDC - 1))

        norm_row = persist.tile([1, N], fp32, tag="norm_row")
        nc.scalar.activation(norm_row[:], sumsq_ps[:], ACT.Sqrt)
        # warm the sigmoid act table right after the only sqrt-table use;
        # reads norm_row so the scheduler cannot hoist it before the Sqrt
        dummy = ssb.tile([1, 1], fp32, tag="dummy")
        nc.scalar.activation(dummy[:], norm_row[0:1, 0:1], ACT.Sigmoid)
        inv_row = persist.tile([1, N], fp32, tag="inv_row")
        nc.vector.reciprocal_approx_fast(inv_row[:], norm_row[:])

        # G to SBUF on the (idle) DVE so the R STT has only one PSUM operand
        g_sb = ssb.tile([RPC, N], bf16, tag="g_sb")
        nc.vector.tensor_copy(g_sb[:], g_ps[:])
        # row invs for rows 0..RPC-1 via PE transpose of inv_row[0, :RPC]
        inv0_ps = spsum.tile([RPC, 1], fp32, tag="inv0_ps")
        nc.tensor.transpose(inv0_ps[:], inv_row[0:1, 0:RPC], one_1x1[:])
        # fold -1000 (the -1/K sigmoid scale) into the row invs: R then
        # holds -1000 * cos-sim, so gtmp's accumulator IS the sigmoid bias
        inv0 = ssb.tile([RPC, 1], fp32, tag="inv0")
        nc.vector.tensor_scalar_mul(inv0[:], inv0_ps[:], -KINV)
        # inv broadcast to RPC partitions (bf16 single-pass matmul; fp32
        # would be two ~1us LOW/HIGH passes)
        inv_row_b = ssb.tile([1, N], bf16, tag="inv_row_b")
        nc.vector.tensor_copy(inv_row_b[:], inv_row[:])
        ib_ps = spsum.tile([RPC, N], fp32, tag="ib_ps")
        nc.tensor.matmul(ib_ps[:], ones_64b[:], inv_row_b[:],
                         start=True, stop=True)

        # R = diag(inv) G diag(inv), stored bf16 for the replication matmuls
        R = persist.tile([RPC, N], bf16, tag="R")
        nc.vector.scalar_tensor_tensor(R[:], g_sb[:], inv0[:], ib_ps[:],
                                       op0=ALU.mult, op1=ALU.mult)

        # --- main: one [128, N] block per pair-bin ---
        bias_flat = persist.tile([128, nblk], fp32, tag="bias_flat")
        den_flat = persist.tile([128, nblk], fp32, tag="den_flat")
        acc_flat = persist.tile([128, nblk], fp32, tag="acc_flat")
        setup_ctx.close()
        s_pool = ctx.enter_context(tc.tile_pool(name="s", bufs=3))
        rp_pool = ctx.enter_context(tc.tile_pool(name="rp", bufs=1, space="PSUM"))
        gp_pool = ctx.enter_context(tc.tile_pool(name="gp", bufs=2, space="PSUM"))

        # all replication matmuls first: PE runs ahead so the DVE/ACT block
        # chains never wait on it
        rreps = []
        for b in range(nblk):
            rrep = rp_pool.tile([128, N], fp32, tag=f"rrep{b}", name=f"rrep{b}")
            nc.tensor.matmul(rrep[:], rep[:, 128 * b:128 * (b + 1)],
                             R[:], start=True, stop=True)
            rreps.append(rrep)
        for b in range(nblk):
            rrep = rreps[b]
            # bias[p] = -1000*R[row(p), sel(p)] via fused iota==sel
            # multiply-accumulate (rrep already carries the -1000 scale)
            tmp = s_pool.tile([128, N], bf16, tag="gtmp")
            nc.vector.scalar_tensor_tensor(
                tmp[:], iota_f[:], m32[:, b:b + 1], rrep[:],
                op0=ALU.is_equal, op1=ALU.mult,
                accum_out=bias_flat[:, b:b + 1])
            sp = s_pool.tile([128, N], bf16, tag="sp")
            nc.scalar.activation(sp[:], rrep[:], ACT.Sigmoid,
                                 bias=bias_flat[:, b:b + 1], scale=-1.0,
                                 accum_out=den_flat[:, b:b + 1])
            # acc from positive-positive pairs: gather bias values of the
            # same row's slots with a block-diagonal selector matmul. With
            # K=0.001 the pair-pair sigmoid is a step to within 5e-5 except
            # at near-ties, so count rg_s' > rg_p directly on the DVE:
            # g2 < bias  <=>  -1000*rg_s' < -1000*rg_p  <=>  rg_s' > rg_p.
            rh = s_pool.tile([128, SLOTS], bf16, tag="rh")
            nc.gpsimd.tensor_scalar(rh[:], m16[:, IBS0 + SLOTS * b:IBS0 + SLOTS * (b + 1)],
                                    bias_flat[:, b:b + 1], None, op0=ALU.mult)
            g2 = gp_pool.tile([128, SLOTS], fp32, tag="g2")
            nc.tensor.matmul(g2[:], m16[:, BDGS0 + 128 * b:BDGS0 + 128 * (b + 1)],
                             rh[:], start=True, stop=True)
            sacc = s_pool.tile([128, SLOTS], bf16, tag="sacc")
            nc.vector.scalar_tensor_tensor(
                sacc[:], g2[:], bias_flat[:, b:b + 1],
                m16[:, MG0 + SLOTS * b:MG0 + SLOTS * (b + 1)],
                op0=ALU.is_lt, op1=ALU.mult,
                accum_out=acc_flat[:, b:b + 1])

        # --- epilogue: prec, weighted global sum ---
        # the self column's sigmoid(1000*(1-rg)) is 1.0 to fp32 for any
        # positive with cos-sim < 0.99, so den_adj = den + 0.5 - 1.0
        ep = ctx.enter_context(tc.tile_pool(name="ep", bufs=1))
        den_adj = ep.tile([128, nblk], fp32, tag="den_adj")
        nc.vector.tensor_scalar_add(den_adj[:], den_flat[:], -0.5)
        recip = ep.tile([128, nblk], fp32, tag="recip")
        nc.vector.reciprocal_approx_fast(recip[:], den_adj[:])
        # step-acc misses the own-slot 0.5 that the sigmoid form counted,
        # so the reference's +1 (minus that 0.5 once) folds to +1.0 here
        acc_w = ep.tile([128, nblk], fp32, tag="acc_w")
        nc.vector.scalar_tensor_tensor(acc_w[:], acc_flat[:], 1.0,
                                       m32[:, nblk:2 * nblk],
                                       op0=ALU.add, op1=ALU.mult)
        pw = ep.tile([128, nblk], fp32, tag="pw")
        nc.vector.tensor_mul(pw[:], acc_w[:], recip[:])
        nsum = ep.tile([128, 1], fp32, tag="nsum")
        nc.vector.tensor_reduce(nsum[:], pw[:], axis=AX.X, op=ALU.add)
        red = gp_pool.tile([1, 1], fp32, tag="red", bufs=1)
        nc.tensor.matmul(red[:], nsum[:], ones_red[:], start=True, stop=True)
        out_sb = ep.tile([1, 1], fp32, tag="out_sb")
        nc.vector.tensor_copy(out_sb[:], red[:])
        nc.sync.dma_start(out_dram, out_sb[:])

    nc.compile()
    return nc


def make_in_maps(query: np.ndarray, target: np.ndarray):
    """Host-side sharding + pair-packing metadata (per-core rolled copies)."""
    import ml_dtypes

    query = np.ascontiguousarray(np.asarray(query), dtype=np.float32)
    tgt = np.asarray(target).reshape(-1)

    # balance rows across cores by positive-pair count (any assignment is
    # valid: each core sees a full permuted copy with its rows first)
    npos_all = np.array([np.sum(tgt == tgt[i]) - 1 for i in range(N)])
    ncnt = int(np.sum(npos_all > 0))
    loads = [0] * NCORES
    assign = [[] for _ in range(NCORES)]
    for i in sorted(range(N), key=lambda i: -npos_all[i]):
        cands = [c for c in range(NCORES) if len(assign[c]) < RPC]
        c = min(cands, key=lambda c: loads[c])
        assign[c].append(i)
        loads[c] += int(npos_all[i])

    cores = []
    for c in range(NCORES):
        mine = assign[c]
        others = [i for i in range(N) if i not in set(mine)]
        perm = np.array(mine + others)
        t_r = tgt[perm]
        rows = []  # per row: positive indices (in permuted coords)
        for q in range(RPC):
            pos = np.flatnonzero(t_r == t_r[q])
            pos = pos[pos != q]
            assert len(pos) <= SLOTS, f"npos {len(pos)} > SLOTS {SLOTS}"
            rows.append(pos)
        # bin-pack rows (row-atomic, best-fit decreasing) into <=128-pair bins
        blocks = []
        fill = []
        order = sorted((q for q in range(RPC) if len(rows[q]) > 0),
                       key=lambda q: -len(rows[q]))
        for q in order:
            npos = len(rows[q])
            best = -1
            for i, f in enumerate(fill):
                if f + npos <= 128 and (best < 0 or f > fill[best]):
                    best = i
            if best < 0:
                blocks.append([q])
                fill.append(npos)
            else:
                blocks[best].append(q)
                fill[best] += npos
        cores.append((perm, rows, blocks))
    nblk = max(len(b) for _, _, b in cores)

    in_maps = []
    for perm, rows, blocks in cores:
        q_r = query[perm]                      # [N, D]
        qt = q_r.T                             # [D, N]
        qt2 = np.ascontiguousarray(
            qt.reshape(D // 128, 128, N).transpose(1, 0, 2).reshape(128, -1)
        ).astype(ml_dtypes.bfloat16)

        M16 = 160 * nblk
        m16 = np.zeros((128, M16), dtype=np.float32)
        repm = np.zeros((RPC, 128 * nblk), dtype=np.float32)
        m32 = np.zeros((128, 2 * nblk), dtype=np.float32)
        m32[:, 0:nblk] = -1.0                  # sel default: matches no iota
        BDGS0, IBS0, MG0 = 0, 128 * nblk, 144 * nblk
        for b, rowlist in enumerate(blocks):
            p = 0
            for q in rowlist:
                npos = len(rows[q])
                pr = range(p, p + npos)
                for s, j in enumerate(rows[q]):
                    m32[p + s, b] = float(j)                 # sel
                    m32[p + s, nblk + b] = 1.0 / npos        # w
                    m16[p + s, IBS0 + SLOTS * b + s] = 1.0   # ibs
                    m16[p + s, MG0 + SLOTS * b:MG0 + SLOTS * b + npos] = 1.0
                for k in pr:
                    for p2 in pr:
                        m16[k, BDGS0 + 128 * b + p2] = 1.0   # bdgs
                    repm[q, 128 * b + k] = 1.0               # rep
                p += npos
        in_maps.append({
            "qt2": qt2,
            "rep": repm.astype(ml_dtypes.bfloat16),
            "m16": m16.astype(ml_dtypes.bfloat16),
            "m32": m32,
        })
    return in_maps, nblk, ncnt


_NC_CACHE = {}


def kernel(query: np.ndarray, target: np.ndarray) -> np.ndarray:
    from concourse import bass_utils

    in_maps, nblk, ncnt = make_in_maps(query, target)
    global _NC_CACHE
    if nblk not in _NC_CACHE:
        _NC_CACHE[nblk] = _build_program(nblk)
    nc = _NC_CACHE[nblk]

    res = bass_utils.run_bass_kernel_spmd(nc, in_maps, core_ids=list(range(NCORES)))
    num = 0.0
    for c in range(NCORES):
        num += float(res.results[c]["out"].reshape(-1)[0])
    mean_ap = num / max(float(ncnt), 1.0)
    return np.float32(1.0 - mean_ap)


# revision 34
# speedup vs baseline: 1.0001x; 1.0001x over previous
"""Trainium2 Bass kernel for nn_MAPLoss (smooth-AP loss, N=512, D=256, K=0.001).

v6: bf16 PE pipeline + minimal-DMA layout. The loss reads prec[i] only at
positive (query, item) pairs (~3600 of 512*511), so each core evaluates just
its ~450 pairs, bin-packed row-atomically into [128 x 512] blocks:
  - norms via DVE square + ones-matmul (no per-chunk ACT chain),
    Sqrt on the one act table that also holds Square, then
    reciprocal_approx_fast on DVE; inv broadcast via a fp32r matmul and
    the 64 row-invs via a PE transpose,
  - R (normalized Gram) built with one fused scalar_tensor_tensor and
    stored bf16; per block a bf16 replication matmul (1 cycle/row),
  - bias[p] = -1000*rg[p] accumulated directly by a fused STT against a
    gpsimd-precomputed onehot(sel) mask; the same bias drives both the
    row sigmoid (den, via accum_out) and the pair-pair sigmoid (acc),
  - epilogue: prec = (acc+0.5)/(den+0.5-sigmoid(1000-1000*rg)), weighted
    global sum with host-folded weights w = 1/npos.
Exactly two ACT_TABLE_LOADs (sqrt-set, sigmoid-set), both scheduled while
DMA/PE work proceeds. HBM per core ~550KB: qt in bf16 once, one-hot
metadata in bf16, iota generated on device.
Host passes only index metadata (selectors, one-hots, weights) derived
from `target`; all float FLOPs run on device. Each core returns its
partial numerator; the host sums and finishes 1 - num/cnt.
"""

import numpy as np
from contextlib import ExitStack

N = 512
D = 256
NCORES = 8
RPC = N // NCORES   # rows per core = 64
SLOTS = 16          # max positives per row (max npos observed is 13)
KINV = 1000.0       # 1/K


def _build_program(nblk):
    import concourse.bacc as bacc
    import concourse.tile as tile
    import concourse.mybir as mybir

    fp32 = mybir.dt.float32
    fp32r = mybir.dt.float32r
    bf16 = mybir.dt.bfloat16
    ALU = mybir.AluOpType
    ACT = mybir.ActivationFunctionType
    AX = mybir.AxisListType

    NDC = D // 128          # 2 dim chunks of qT
    BDGS0 = 0               # meta16 column offsets
    IBS0 = 128 * nblk
    MG0 = 144 * nblk
    M16 = 160 * nblk

    nc = bacc.Bacc("TRN2", target_bir_lowering=False, debug=False,
                   num_devices=NCORES)
    qt2_dram = nc.dram_tensor("qt2", [128, NDC * N], bf16,
                              kind="ExternalInput").ap()
    rep_dram = nc.dram_tensor("rep", [RPC, 128 * nblk], bf16,
                              kind="ExternalInput").ap()
    m16_dram = nc.dram_tensor("m16", [128, M16], bf16,
                              kind="ExternalInput").ap()
    m32_dram = nc.dram_tensor("m32", [128, 2 * nblk], fp32,
                              kind="ExternalInput").ap()
    out_dram = nc.dram_tensor("out", [1, 1], fp32, kind="ExternalOutput").ap()

    with tile.TileContext(nc) as tc, ExitStack() as ctx:
        const = ctx.enter_context(tc.tile_pool(name="const", bufs=1))
        persist = ctx.enter_context(tc.tile_pool(name="persist", bufs=1))
        setup_ctx = ctx.enter_context(ExitStack())
        spsum = setup_ctx.enter_context(
            tc.tile_pool(name="spsum", bufs=1, space="PSUM"))
        ssb = setup_ctx.enter_context(tc.tile_pool(name="ssb", bufs=1))

        # --- constants (gpsimd) + the three input DMAs on separate rings ---
        ones_colb = const.tile([128, 1], bf16, tag="ones_colb")
        nc.gpsimd.memset(ones_colb[:], 1.0)
        ones_64b = const.tile([1, RPC], bf16, tag="ones_64b")
        nc.gpsimd.memset(ones_64b[:], 1.0)
        ones_red = const.tile([128, 1], fp32, tag="ones_red")
        nc.gpsimd.memset(ones_red[:], 1.0)
        one_1x1 = const.tile([1, 1], fp32, tag="one_1x1")
        nc.gpsimd.memset(one_1x1[:], 1.0)
        k1000 = const.tile([128, 1], fp32, tag="k1000")
        nc.gpsimd.memset(k1000[:], KINV)
        iota_f = const.tile([128, N], fp32, tag="iota_f")
        nc.gpsimd.iota(iota_f[:], pattern=[[1, N]], base=0,
                       channel_multiplier=0,
                       allow_small_or_imprecise_dtypes=True)

        # DMA throughput is packet-bound (~45ns per partition-row packet
        # regardless of 1KB vs 2KB), so keep full 2KB rows and split by
        # partition range across two queues.
        qt2 = persist.tile([128, NDC * N], bf16, tag="qt2")
        nc.sync.dma_start(qt2[0:64, :], qt2_dram[0:64, :])
        nc.scalar.dma_start(qt2[64:128, :], qt2_dram[64:128, :])
        m16 = persist.tile([128, M16], bf16, tag="m16")
        nc.sync.dma_start(m16[0:64, :], m16_dram[0:64, :])
        nc.scalar.dma_start(m16[64:128, :], m16_dram[64:128, :])
        m32 = persist.tile([128, 2 * nblk], fp32, tag="m32")
        nc.gpsimd.dma_start(m32[:], m32_dram)
        rep = persist.tile([RPC, 128 * nblk], bf16, tag="rep")
        nc.gpsimd.dma_start(rep[:], rep_dram)

        # --- norms: sumsq[1,N] = ones^T (qt.^2), via matmul over NDC chunks ---
        qtsq = ssb.tile([128, NDC * N], bf16, tag="qtsq")
        sumsq_ps = spsum.tile([1, N], fp32, tag="sumsq_ps")
        g_ps = spsum.tile([RPC, N], fp32, tag="g_ps")
        for c in range(NDC):
            nc.vector.tensor_mul(qtsq[:, N * c:N * (c + 1)],
                                 qt2[:, N * c:N * (c + 1)],
                                 qt2[:, N * c:N * (c + 1)])
            nc.tensor.matmul(sumsq_ps[:], ones_colb[:],
                             qtsq[:, N * c:N * (c + 1)],
                             start=(c == 0), stop=(c == NDC - 1))
            # Gram for rows 0..RPC-1 (PE, interleaved with the norm matmuls)
            nc.tensor.matmul(g_ps[:], qt2[:, N * c:N * c + RPC],
                             qt2[:, N * c:N * (c + 1)],
                             start=(c == 0), stop=(c == NDC - 1))

        norm_row = persist.tile([1, N], fp32, tag="norm_row")
        nc.scalar.activation(norm_row[:], sumsq_ps[:], ACT.Sqrt)
        # warm the sigmoid act table right after the only sqrt-table use;
        # reads norm_row so the scheduler cannot hoist it before the Sqrt
        dummy = ssb.tile([1, 1], fp32, tag="dummy")
        nc.scalar.activation(dummy[:], norm_row[0:1, 0:1], ACT.Sigmoid)
        inv_row = persist.tile([1, N], fp32, tag="inv_row")
        nc.vector.reciprocal_approx_fast(inv_row[:], norm_row[:])

        # G to SBUF on the (idle) DVE so the R STT has only one PSUM operand
        g_sb = ssb.tile([RPC, N], bf16, tag="g_sb")
        nc.vector.tensor_copy(g_sb[:], g_ps[:])
        # row invs for rows 0..RPC-1 via PE transpose of inv_row[0, :RPC]
        inv0_ps = spsum.tile([RPC, 1], fp32, tag="inv0_ps")
        nc.tensor.transpose(inv0_ps[:], inv_row[0:1, 0:RPC], one_1x1[:])
        # fold -1000 (the -1/K sigmoid scale) into the row invs: R then
        # holds -1000 * cos-sim, so gtmp's accumulator IS the sigmoid bias
        inv0 = ssb.tile([RPC, 1], fp32, tag="inv0")
        nc.vector.tensor_scalar_mul(inv0[:], inv0_ps[:], -KINV)
        # inv broadcast to RPC partitions (bf16 single-pass matmul; fp32
        # would be two ~1us LOW/HIGH passes)
        inv_row_b = ssb.tile([1, N], bf16, tag="inv_row_b")
        nc.vector.tensor_copy(inv_row_b[:], inv_row[:])
        ib_ps = spsum.tile([RPC, N], fp32, tag="ib_ps")
        nc.tensor.matmul(ib_ps[:], ones_64b[:], inv_row_b[:],
                         start=True, stop=True)

        # R = diag(inv) G diag(inv), stored bf16 for the replication matmuls
        R = persist.tile([RPC, N], bf16, tag="R")
        nc.vector.scalar_tensor_tensor(R[:], g_sb[:], inv0[:], ib_ps[:],
                                       op0=ALU.mult, op1=ALU.mult)

        # --- main: one [128, N] block per pair-bin ---
        bias_flat = persist.tile([128, nblk], fp32, tag="bias_flat")
        den_flat = persist.tile([128, nblk], fp32, tag="den_flat")
        acc_flat = persist.tile([128, nblk], fp32, tag="acc_flat")
        den_adj = persist.tile([128, nblk], fp32, tag="den_adj")
        recip_f = persist.tile([128, nblk], fp32, tag="recip_f")
        setup_ctx.close()
        s_pool = ctx.enter_context(tc.tile_pool(name="s", bufs=3))
        rp_pool = ctx.enter_context(tc.tile_pool(name="rp", bufs=1, space="PSUM"))
        gp_pool = ctx.enter_context(tc.tile_pool(name="gp", bufs=1, space="PSUM"))
        g2_all = gp_pool.tile([128, SLOTS * nblk], fp32, tag="g2_all")

        # all replication matmuls first: PE runs ahead so the DVE/ACT block
        # chains never wait on it
        rreps = []
        for b in range(nblk):
            rrep = rp_pool.tile([128, N], fp32, tag=f"rrep{b}", name=f"rrep{b}")
            nc.tensor.matmul(rrep[:], rep[:, 128 * b:128 * (b + 1)],
                             R[:], start=True, stop=True)
            rreps.append(rrep)
        for b in range(nblk):
            rrep = rreps[b]
            # bias[p] = -1000*R[row(p), sel(p)] via fused iota==sel
            # multiply-accumulate (rrep already carries the -1000 scale)
            tmp = s_pool.tile([128, N], bf16, tag="gtmp")
            nc.vector.scalar_tensor_tensor(
                tmp[:], iota_f[:], m32[:, b:b + 1], rrep[:],
                op0=ALU.is_equal, op1=ALU.mult,
                accum_out=bias_flat[:, b:b + 1])
            sp = s_pool.tile([128, N], bf16, tag="sp")
            nc.scalar.activation(sp[:], rrep[:], ACT.Sigmoid,
                                 bias=bias_flat[:, b:b + 1], scale=-1.0,
                                 accum_out=den_flat[:, b:b + 1])
            # acc from positive-positive pairs: gather bias values of the
            # same row's slots with a block-diagonal selector matmul. With
            # K=0.001 the pair-pair sigmoid is a step to within 5e-5 except
            # at near-ties, so count rg_s' > rg_p directly on the DVE:
            # g2 < bias  <=>  -1000*rg_s' < -1000*rg_p  <=>  rg_s' > rg_p.
            rh = s_pool.tile([128, SLOTS], bf16, tag="rh")
            nc.gpsimd.tensor_scalar(rh[:], m16[:, IBS0 + SLOTS * b:IBS0 + SLOTS * (b + 1)],
                                    bias_flat[:, b:b + 1], None, op0=ALU.mult)
            nc.tensor.matmul(g2_all[:, SLOTS * b:SLOTS * (b + 1)],
                             m16[:, BDGS0 + 128 * b:BDGS0 + 128 * (b + 1)],
                             rh[:], start=True, stop=True)
            # den_adj + reciprocal for this block while the DVE has slack
            nc.vector.tensor_scalar_add(den_adj[:, b:b + 1],
                                        den_flat[:, b:b + 1], -0.5)
            nc.vector.reciprocal_approx_fast(recip_f[:, b:b + 1],
                                             den_adj[:, b:b + 1])
        # pair-pair step-counts after the gtmp stream so the DVE never
        # stalls on the gpsimd->PE round trip mid-phase
        for b in range(nblk):
            sacc = s_pool.tile([128, SLOTS], bf16, tag="sacc")
            nc.vector.scalar_tensor_tensor(
                sacc[:], g2_all[:, SLOTS * b:SLOTS * (b + 1)],
                bias_flat[:, b:b + 1],
                m16[:, MG0 + SLOTS * b:MG0 + SLOTS * (b + 1)],
                op0=ALU.is_lt, op1=ALU.mult,
                accum_out=acc_flat[:, b:b + 1])

        # --- epilogue: prec, weighted global sum ---
        # (den_adj = den + 0.5 - s_colg with s_colg==1.0 to fp32, and the
        # step-acc's +1.0, were prepared per block above)
        ep = ctx.enter_context(tc.tile_pool(name="ep", bufs=1))
        acc_w = ep.tile([128, nblk], fp32, tag="acc_w")
        nc.vector.scalar_tensor_tensor(acc_w[:], acc_flat[:], 1.0,
                                       m32[:, nblk:2 * nblk],
                                       op0=ALU.add, op1=ALU.mult)
        pw = ep.tile([128, nblk], fp32, tag="pw")
        nc.vector.tensor_mul(pw[:], acc_w[:], recip_f[:])
        nsum = ep.tile([128, 1], fp32, tag="nsum")
        nc.vector.tensor_reduce(nsum[:], pw[:], axis=AX.X, op=ALU.add)
        red = gp_pool.tile([1, 1], fp32, tag="red", bufs=1)
        nc.tensor.matmul(red[:], nsum[:], ones_red[:], start=True, stop=True)
        out_sb = ep.tile([1, 1], fp32, tag="out_sb")
        nc.vector.tensor_copy(out_sb[:], red[:])
        nc.sync.dma_start(out_dram, out_sb[:])

    nc.compile()
    return nc


def make_in_maps(query: np.ndarray, target: np.ndarray):
    """Host-side sharding + pair-packing metadata (per-core rolled copies)."""
    import ml_dtypes

    query = np.ascontiguousarray(np.asarray(query), dtype=np.float32)
    tgt = np.asarray(target).reshape(-1)

    # balance rows across cores by positive-pair count (any assignment is
    # valid: each core sees a full permuted copy with its rows first)
    npos_all = np.array([np.sum(tgt == tgt[i]) - 1 for i in range(N)])
    ncnt = int(np.sum(npos_all > 0))
    loads = [0] * NCORES
    assign = [[] for _ in range(NCORES)]
    for i in sorted(range(N), key=lambda i: -npos_all[i]):
        cands = [c for c in range(NCORES) if len(assign[c]) < RPC]
        c = min(cands, key=lambda c: loads[c])
        assign[c].append(i)
        loads[c] += int(npos_all[i])

    cores = []
    for c in range(NCORES):
        mine = assign[c]
        others = [i for i in range(N) if i not in set(mine)]
        perm = np.array(mine + others)
        t_r = tgt[perm]
        rows = []  # per row: positive indices (in permuted coords)
        for q in range(RPC):
            pos = np.flatnonzero(t_r == t_r[q])
            pos = pos[pos != q]
            assert len(pos) <= SLOTS, f"npos {len(pos)} > SLOTS {SLOTS}"
            rows.append(pos)
        # bin-pack rows (row-atomic, best-fit decreasing) into <=128-pair bins
        blocks = []
        fill = []
        order = sorted((q for q in range(RPC) if len(rows[q]) > 0),
                       key=lambda q: -len(rows[q]))
        for q in order:
            npos = len(rows[q])
            best = -1
            for i, f in enumerate(fill):
                if f + npos <= 128 and (best < 0 or f > fill[best]):
                    best = i
            if best < 0:
                blocks.append([q])
                fill.append(npos)
            else:
                blocks[best].append(q)
                fill[best] += npos
        cores.append((perm, rows, blocks))
    nblk = max(len(b) for _, _, b in cores)

    in_maps = []
    for perm, rows, blocks in cores:
        q_r = query[perm]                      # [N, D]
        qt = q_r.T                             # [D, N]
        qt2 = np.ascontiguousarray(
            qt.reshape(D // 128, 128, N).transpose(1, 0, 2).reshape(128, -1)
        ).astype(ml_dtypes.bfloat16)

        M16 = 160 * nblk
        m16 = np.zeros((128, M16), dtype=np.float32)
        repm = np.zeros((RPC, 128 * nblk), dtype=np.float32)
        m32 = np.zeros((128, 2 * nblk), dtype=np.float32)
        m32[:, 0:nblk] = -1.0                  # sel default: matches no iota
        BDGS0, IBS0, MG0 = 0, 128 * nblk, 144 * nblk
        for b, rowlist in enumerate(blocks):
            p = 0
            for q in rowlist:
                npos = len(rows[q])
                pr = range(p, p + npos)
                for s, j in enumerate(rows[q]):
                    m32[p + s, b] = float(j)                 # sel
                    m32[p + s, nblk + b] = 1.0 / npos        # w
                    m16[p + s, IBS0 + SLOTS * b + s] = 1.0   # ibs
                    m16[p + s, MG0 + SLOTS * b:MG0 + SLOTS * b + npos] = 1.0
                for k in pr:
                    for p2 in pr:
                        m16[k, BDGS0 + 128 * b + p2] = 1.0   # bdgs
                    repm[q, 128 * b + k] = 1.0               # rep
                p += npos
        in_maps.append({
            "qt2": qt2,
            "rep": repm.astype(ml_dtypes.bfloat16),
            "m16": m16.astype(ml_dtypes.bfloat16),
            "m32": m32,
        })
    return in_maps, nblk, ncnt


_NC_CACHE = {}


def kernel(query: np.ndarray, target: np.ndarray) -> np.ndarray:
    from concourse import bass_utils

    in_maps, nblk, ncnt = make_in_maps(query, target)
    global _NC_CACHE
    if nblk not in _NC_CACHE:
        _NC_CACHE[nblk] = _build_program(nblk)
    nc = _NC_CACHE[nblk]

    res = bass_utils.run_bass_kernel_spmd(nc, in_maps, core_ids=list(range(NCORES)))
    num = 0.0
    for c in range(NCORES):
        num += float(res.results[c]["out"].reshape(-1)[0])
    mean_ap = num / max(float(ncnt), 1.0)
    return np.float32(1.0 - mean_ap)
